# revision 1
# baseline (speedup 1.0000x reference)
"""Trainium2 Bass kernel for nn_CLoss_68521908241007 (retrieval_knn).

Math (per the reference):
  sq_dist[i,j] = ||feat_i||^2 + ||feat2_j||^2 - 2 feat_i . feat2_j
  logits = -temp * sqrt(sq_dist)
  loss = mean_i( logsumexp_j(logits[i,:]) - logits[i, labels_i] )

Sharding: feat rows split across 8 cores (1024 queries each); feat2 replicated.
Each core computes its 1024x8192 block and returns per-row losses; the host
concatenates and takes the mean (the "all-reduce").

Per-core pipeline (PE clock is capped at 1.2 GHz on this system, so PE work
is kept to the bare G matmuls):
  - PE (bf16): psum = G = featT.T @ feat2T      [4x 512-col matmuls per group]
  - DVE: dist_buf = bf16(psum + ybc)            [drains PSUM, adds the y term]
         where ybc = -0.5*(y_sq - 128) broadcast on all 128 partitions
  - ACT: dist = Sqrt(-2*dist_buf + (x_sq+128))  [one 8K-wide call per q-block]
         e    = Exp(-temp*dist)                 [in place, fused row-sum]
  - ACT ops run in two sqrt->exp macro phases; PE/DVE stream the second
    half's matmuls during the first exp phase. Table switches cost ~2.7us,
    so phases are serialized on ACT via a data-chained zero bias (zrow).
  - y_sq broadcast comes out of an all-ones 128x128 reduce matmul (every
    output partition gets the column norms), shifted/scaled by one DVE op.
"""

import numpy as np
from contextlib import ExitStack

import concourse.bass as bass
import concourse.bacc as bacc
import concourse.mybir as mybir
import concourse.tile as tile
from concourse.bass_utils import run_bass_kernel_spmd

AF = mybir.ActivationFunctionType
ALU = mybir.AluOpType
AX = mybir.AxisListType
f32 = mybir.dt.float32
bf16 = mybir.dt.bfloat16

N_CORES = 8
N, M, D = 8192, 8192, 128
NQ = N // N_CORES        # queries per core
QB = NQ // 128           # q-blocks per core (8)
KSEG = 512               # keys per matmul
NKSEG = M // KSEG        # 16
GRP = 4                  # k-segs per psum group (4 banks)
NGRP = NKSEG // GRP      # 4 groups per q-block
HALF = QB // 2           # q-blocks per ACT macro phase


def _body(tc, out_d, featT_d, featn_d, feat2T_d, sel_d, temp_d):
    nc = tc.nc
    with ExitStack() as ctx:
        singles = ctx.enter_context(tc.tile_pool(name="singles", bufs=1))
        sqp = ctx.enter_context(tc.tile_pool(name="sqp", bufs=4))
        distp = ctx.enter_context(tc.tile_pool(name="distp", bufs=QB))
        psp = ctx.enter_context(tc.tile_pool(name="psp", bufs=2, space="PSUM"))
        smallp = ctx.enter_context(tc.tile_pool(name="smallp", bufs=2))

        # ---- inputs -> SBUF; feat2T first: it heads the critical y_sq chain
        feat2T_sb = singles.tile([D, M], bf16)
        for c in range(4):
            w = M // 4
            nc.sync.dma_start(out=feat2T_sb[:, c * w:(c + 1) * w],
                              in_=feat2T_d[:, c * w:(c + 1) * w])
        featT_sb = singles.tile([D, NQ], bf16)
        nc.sync.dma_start(out=featT_sb, in_=featT_d)
        featn_sb = singles.tile([128, QB, D], bf16)
        nc.sync.dma_start(out=featn_sb,
                          in_=featn_d.rearrange("(b p) d -> p b d", p=128))
        sel_sb = singles.tile([128, QB, D], bf16)
        nc.sync.dma_start(out=sel_sb,
                          in_=sel_d.rearrange("(b p) d -> p b d", p=128))
        pos_temp = singles.tile([128, 1], f32)
        nc.sync.dma_start(out=pos_temp, in_=temp_d.to_broadcast((128, 1)))

        # ---- constants
        ones_mat_f = singles.tile([D, 128], f32)
        nc.vector.memset(ones_mat_f, 1.0)
        ones_mat = singles.tile([D, 128], bf16)
        nc.vector.tensor_copy(ones_mat, ones_mat_f)
        neg_temp = singles.tile([128, 1], f32)
        nc.vector.tensor_scalar_mul(neg_temp, pos_temp, -1.0)

        # ---- ybc[128, M] = bf16(-0.5*(y_sq - 128)) on every partition.
        # The all-ones 128x128 reduce matmul broadcasts the column norms to
        # all output partitions directly in PSUM; one DVE tensor_scalar per
        # 4-bank chunk shifts+scales it into SBUF.
        # Squares run on ACT (Square is in every activation-table set, and
        # ACT is otherwise idle until the first sqrt) so DVE's pre-qb0
        # critical chain is just the ybc shifts + qb0 drains.
        ybc = singles.tile([128, M], bf16)
        for g in range(NGRP):
            ps_y = psp.tile([128, GRP * KSEG], f32, tag="ps")
            for si in range(GRP):
                s = g * GRP + si
                sq = sqp.tile([128, KSEG], bf16, tag="sq")
                nc.scalar.activation(
                    out=sq, in_=feat2T_sb[:, s * KSEG:(s + 1) * KSEG],
                    func=AF.Square, bias=0.0, scale=1.0)
                nc.tensor.matmul(ps_y[:, si * KSEG:(si + 1) * KSEG],
                                 lhsT=ones_mat, rhs=sq, start=True, stop=True)
            nc.vector.tensor_scalar(
                out=ybc[:, g * GRP * KSEG:(g + 1) * GRP * KSEG],
                in0=ps_y, scalar1=-128.0, scalar2=-0.5,
                op0=ALU.add, op1=ALU.mult)

        # ---- x_sq (+128 shift) for the sqrt bias -- ACT Square with fused
        # row-sum (also in ACT's idle startup window)
        x_sq = singles.tile([128, QB], f32)
        for b in range(QB):
            fsq = smallp.tile([128, D], f32, tag="fsq")
            nc.scalar.activation(out=fsq, in_=featn_sb[:, b, :],
                                 func=AF.Square, bias=0.0, scale=1.0,
                                 accum_out=x_sq[:, b:b + 1])
        xb = singles.tile([128, QB], f32)
        nc.vector.tensor_scalar_add(xb, x_sq, 128.0)

        # ---- picked-label squared distance (DVE, early: ACT needs pdist in
        # the last sqrt-table window)
        psq = singles.tile([128, QB], f32)
        diff_all = singles.tile([128, QB, D], f32)
        nc.vector.tensor_sub(diff_all, featn_sb, sel_sb)
        for b in range(QB):
            dsq = smallp.tile([128, D], f32, tag="fsq")
            nc.vector.tensor_mul(dsq, diff_all[:, b, :], diff_all[:, b, :])
            nc.vector.reduce_sum(psq[:, b:b + 1], dsq, axis=AX.X)

        # ---- main pipeline, emitted in pair-of-qb chunks so every engine's
        # priority queue interleaves: [mains+drains x2qb][sqrt x2][zrow]
        # [exp x2] ... ACT table phases are data-chained in both directions
        # (zrow: exp after last sqrt of the pair; m2: sqrt of pair h after
        # exp of pair h-1) so the ~2.7us table reloads stay at 2 per pair.
        NPH = 4
        PER = QB // NPH
        S = singles.tile([128, QB], f32)
        pdist = singles.tile([128, QB], f32)
        zrows = singles.tile([128, NPH], f32)
        m2s = singles.tile([128, NPH], f32)
        dist_tiles = []
        for h in range(NPH):
            qbs = range(h * PER, (h + 1) * PER)
            for b in qbs:
                dist_t = distp.tile([128, M], bf16, tag="dist")
                dist_tiles.append(dist_t)
                for g in range(NGRP):
                    ps = psp.tile([128, GRP * KSEG], f32, tag="ps")
                    for si in range(GRP):
                        nc.tensor.matmul(
                            ps[:, si * KSEG:(si + 1) * KSEG],
                            lhsT=featT_sb[:, b * 128:(b + 1) * 128],
                            rhs=feat2T_sb[:, (g * GRP + si) * KSEG:
                                          (g * GRP + si + 1) * KSEG],
                            start=True, stop=True)
                    nc.vector.tensor_add(
                        dist_t[:, g * GRP * KSEG:(g + 1) * GRP * KSEG],
                        ps, ybc[:, g * GRP * KSEG:(g + 1) * GRP * KSEG])
            if h == 0:
                scale_h = -2.0
            else:
                nc.vector.tensor_scalar(
                    out=m2s[:, h:h + 1], in0=S[:, h * PER - 1:h * PER],
                    scalar1=0.0, scalar2=-2.0, op0=ALU.mult, op1=ALU.add)
                scale_h = m2s[:, h:h + 1]
            for b in qbs:
                nc.scalar.activation(
                    out=dist_tiles[b], in_=dist_tiles[b], func=AF.Sqrt,
                    bias=xb[:, b:b + 1], scale=scale_h)
            if h == NPH - 1:
                # picked-label distance; still inside a sqrt-table window
                nc.scalar.activation(out=pdist, in_=psq, func=AF.Sqrt,
                                     bias=0.0, scale=1.0)
                nc.vector.tensor_scalar_mul(zrows[:, h:h + 1],
                                            pdist[:, 0:1], 0.0)
            else:
                last = (h + 1) * PER - 1
                nc.vector.tensor_scalar_mul(zrows[:, h:h + 1],
                                            dist_tiles[last][:, M - 1:M], 0.0)
            for b in qbs:
                nc.scalar.activation(
                    out=dist_tiles[b], in_=dist_tiles[b], func=AF.Exp,
                    bias=zrows[:, h:h + 1], scale=neg_temp[:, 0:1],
                    accum_out=S[:, b:b + 1])

        # ---- finals: loss_row = Ln(S) + temp * pdist
        logz = singles.tile([128, QB], f32)
        nc.scalar.activation(out=logz, in_=S, func=AF.Ln, bias=0.0, scale=1.0)
        picked = singles.tile([128, QB], f32)
        nc.vector.tensor_scalar_mul(picked, pdist, pos_temp[:, 0:1])
        loss_t = singles.tile([128, QB], f32)
        nc.vector.tensor_add(loss_t, picked, logz)
        nc.sync.dma_start(out=out_d, in_=loss_t)


def build_program():
    nc = bacc.Bacc("TRN2", target_bir_lowering=False, debug=False,
                   num_devices=N_CORES)
    featT = nc.dram_tensor("featT", [D, NQ], bf16, kind="ExternalInput").ap()
    featn = nc.dram_tensor("featn", [NQ, D], bf16, kind="ExternalInput").ap()
    feat2T = nc.dram_tensor("feat2T", [D, M], bf16, kind="ExternalInput").ap()
    sel = nc.dram_tensor("sel", [NQ, D], bf16, kind="ExternalInput").ap()
    temp = nc.dram_tensor("temp", [1, 1], f32, kind="ExternalInput").ap()
    out = nc.dram_tensor("out", [128, QB], f32, kind="ExternalOutput").ap()
    with tile.TileContext(nc) as tc:
        _body(tc, out, featT, featn, feat2T, sel, temp)
    nc.compile()
    return nc


def make_in_maps(feat, feat2, temp, labels):
    import ml_dtypes
    feat = np.ascontiguousarray(np.asarray(feat, dtype=np.float32))
    feat2 = np.ascontiguousarray(np.asarray(feat2, dtype=np.float32))
    labels_np = np.asarray(labels).astype(np.int64)
    temp_np = np.asarray(temp, dtype=np.float32).reshape(1, 1)
    feat2T = np.ascontiguousarray(feat2.T).astype(ml_dtypes.bfloat16)
    sel_full = feat2[labels_np]
    in_maps = []
    for c in range(N_CORES):
        fs = feat[c * NQ:(c + 1) * NQ]
        in_maps.append({
            "featT": np.ascontiguousarray(fs.T).astype(ml_dtypes.bfloat16),
            "featn": fs.astype(ml_dtypes.bfloat16),
            "feat2T": feat2T,
            "sel": np.ascontiguousarray(sel_full[c * NQ:(c + 1) * NQ]).astype(ml_dtypes.bfloat16),
            "temp": temp_np,
        })
    return in_maps


def combine_outputs(per_core_outs):
    # out[p, b] is the loss for query q = b*128 + p of that core's shard
    rows = [np.asarray(o).T.reshape(-1) for o in per_core_outs]
    return np.float32(np.concatenate(rows).mean())


_PROGRAM = None


def kernel(feat, feat2, temp, labels):
    global _PROGRAM
    if _PROGRAM is None:
        _PROGRAM = build_program()
    in_maps = make_in_maps(feat, feat2, temp, labels)
    res = run_bass_kernel_spmd(_PROGRAM, in_maps, core_ids=list(range(N_CORES)))
    return combine_outputs([r["out"] for r in res.results])



# revision 2
# speedup vs baseline: 1.1485x; 1.1485x over previous
"""Trainium2 Bass kernel for nn_CLoss_68521908241007 (retrieval_knn).

Math (per the reference):
  sq_dist[i,j] = ||feat_i||^2 + ||feat2_j||^2 - 2 feat_i . feat2_j
  logits = -temp * sqrt(sq_dist)
  loss = mean_i( logsumexp_j(logits[i,:]) - logits[i, labels_i] )

Sharding: feat rows split across 8 cores (1024 queries each); feat2 replicated.
Each core returns S[p,b] = sum_j exp(-temp*dist) for query q=b*128+p of its
shard; the host finishes with ln(S) + temp*||x_q - y_label|| and the mean.

Per-core pipeline:
  - PE (bf16): psum = G = featT.T @ feat2T          [4x 512-col matmuls/group]
  - DVE: dist_buf = bf16(psum + ybc)                [drain + y^2 term]
         ybc = -0.5*(y_sq - 128) comes precomputed from the host
  - ACT: dist = Sqrt(-2*dist_buf + xb)              [xb = x_sq + 128, host]
  - exp + row-sum, split across engines to balance ACT vs DVE:
      * A_QBS qblocks: ACT Exp(scale=-temp) with fused accum row-sum
      * S_QBS qblocks: DVE Schraudolph exp: w = int32(dist*(-temp*2^23/ln2)
        + B); bits(w) viewed as f32 are exp(-temp*dist); row-sum via DVE
        tensor_reduce on the bitcast view. (C=486411 mean-zero constant,
        calibrated offline; final loss rel-err ~1e-3 vs 2e-2 budget.)
  - ACT table phases grouped (sqrt x4, exp x4) to amortize ~2.7us table loads.
"""

import numpy as np
from contextlib import ExitStack

import concourse.bass as bass
import concourse.bacc as bacc
import concourse.mybir as mybir
import concourse.tile as tile
from concourse.bass_utils import run_bass_kernel_spmd

AF = mybir.ActivationFunctionType
ALU = mybir.AluOpType
AX = mybir.AxisListType
f32 = mybir.dt.float32
bf16 = mybir.dt.bfloat16
i32 = mybir.dt.int32

N_CORES = 8
N, M, D = 8192, 8192, 128
NQ = N // N_CORES        # queries per core
QB = NQ // 128           # q-blocks per core (8)
KSEG = 512               # keys per matmul
GRP = 4                  # k-segs per psum group (4 banks)
NGRP = (M // KSEG) // GRP  # 4 groups per q-block

S_QBS = 2                # trailing qblocks whose exp+sum runs on DVE
A_EXP = 2.0 ** 23 / np.log(2.0)
B_EXP = float((127 << 23) - 486411)


def _emit_qb_main(nc, psp, dist_t, featT_sb, feat2T_sb, ybc_sb, b):
    """Matmuls + DVE drain-add for one q-block; dist_t gets x.y - y_sq/2 + 64."""
    for g in range(NGRP):
        ps = psp.tile([128, GRP * KSEG], f32, tag="ps")
        for si in range(GRP):
            s = g * GRP + si
            nc.tensor.matmul(
                ps[:, si * KSEG:(si + 1) * KSEG],
                lhsT=featT_sb[:, b * 128:(b + 1) * 128],
                rhs=feat2T_sb[:, s * KSEG:(s + 1) * KSEG],
                start=True, stop=True)
        nc.vector.tensor_add(
            dist_t[:, g * GRP * KSEG:(g + 1) * GRP * KSEG],
            ps, ybc_sb[:, g * GRP * KSEG:(g + 1) * GRP * KSEG])


def _body(tc, out_d, featT_d, feat2T_d, ybc_d, xb_d, negt_d, aexp_d):
    nc = tc.nc
    with ExitStack() as ctx:
        singles = ctx.enter_context(tc.tile_pool(name="singles", bufs=1))
        distp = ctx.enter_context(tc.tile_pool(name="distp", bufs=5))
        psp = ctx.enter_context(tc.tile_pool(name="psp", bufs=2, space="PSUM"))

        # ---- inputs -> SBUF; feat2T first (heads the first matmul chain)
        feat2T_sb = singles.tile([D, M], bf16)
        for c in range(4):
            w = M // 4
            nc.sync.dma_start(out=feat2T_sb[:, c * w:(c + 1) * w],
                              in_=feat2T_d[:, c * w:(c + 1) * w])
        featT_sb = singles.tile([D, NQ], bf16)
        nc.sync.dma_start(out=featT_sb, in_=featT_d)
        ybc_sb = singles.tile([128, M], bf16)
        for c in range(4):
            w = M // 4
            nc.sync.dma_start(out=ybc_sb[:, c * w:(c + 1) * w],
                              in_=ybc_d[:, c * w:(c + 1) * w])
        xb = singles.tile([128, QB], f32)
        nc.sync.dma_start(out=xb, in_=xb_d)
        negt = singles.tile([128, 1], f32)
        nc.sync.dma_start(out=negt, in_=negt_d)
        aexp = singles.tile([128, 1], f32)
        nc.sync.dma_start(out=aexp, in_=aexp_d)

        S = singles.tile([128, QB], f32)
        w_t = singles.tile([128, M], i32)   # Schraudolph scratch (1 buf)
        zrows = singles.tile([128, 2], f32)
        dist_tiles = [None] * QB

        # ---- phase pair 1: qb 0..3 sqrt, then qb 0..3 exp on ACT
        HALF = QB // 2
        for b in range(HALF):
            dist_t = distp.tile([128, M], bf16, tag="dist")
            dist_tiles[b] = dist_t
            _emit_qb_main(nc, psp, dist_t, featT_sb, feat2T_sb, ybc_sb, b)
            nc.scalar.activation(out=dist_t, in_=dist_t, func=AF.Sqrt,
                                 bias=xb[:, b:b + 1], scale=-2.0)
        nc.vector.tensor_scalar_mul(zrows[:, 0:1],
                                    dist_tiles[HALF - 1][:, M - 1:M], 0.0)
        for b in range(HALF):
            nc.scalar.activation(
                out=dist_tiles[b], in_=dist_tiles[b], func=AF.Exp,
                bias=zrows[:, 0:1], scale=negt[:, 0:1],
                accum_out=S[:, b:b + 1])

        # ---- phase pair 2: qb 4..7 sqrt, then exp (ACT) / Schraudolph (DVE)
        for b in range(HALF, QB):
            dist_t = distp.tile([128, M], bf16, tag="dist")
            dist_tiles[b] = dist_t
            _emit_qb_main(nc, psp, dist_t, featT_sb, feat2T_sb, ybc_sb, b)
            nc.scalar.activation(out=dist_t, in_=dist_t, func=AF.Sqrt,
                                 bias=xb[:, b:b + 1], scale=-2.0)
        nc.vector.tensor_scalar_mul(zrows[:, 1:2],
                                    dist_tiles[QB - 1][:, M - 1:M], 0.0)
        for b in range(HALF, QB - S_QBS):
            nc.scalar.activation(
                out=dist_tiles[b], in_=dist_tiles[b], func=AF.Exp,
                bias=zrows[:, 1:2], scale=negt[:, 0:1],
                accum_out=S[:, b:b + 1])
        for b in range(QB - S_QBS, QB):
            nc.vector.tensor_scalar(
                out=w_t, in0=dist_tiles[b], scalar1=aexp[:, 0:1],
                scalar2=B_EXP, op0=ALU.mult, op1=ALU.add)
            nc.vector.tensor_reduce(
                out=S[:, b:b + 1], in_=w_t.bitcast(f32),
                axis=AX.X, op=ALU.add)

        nc.sync.dma_start(out=out_d, in_=S)


def build_program():
    nc = bacc.Bacc("TRN2", target_bir_lowering=False, debug=False,
                   num_devices=N_CORES)
    featT = nc.dram_tensor("featT", [D, NQ], bf16, kind="ExternalInput").ap()
    feat2T = nc.dram_tensor("feat2T", [D, M], bf16, kind="ExternalInput").ap()
    ybc = nc.dram_tensor("ybc", [128, M], bf16, kind="ExternalInput").ap()
    xb = nc.dram_tensor("xb", [128, QB], f32, kind="ExternalInput").ap()
    negt = nc.dram_tensor("negt", [128, 1], f32, kind="ExternalInput").ap()
    aexp = nc.dram_tensor("aexp", [128, 1], f32, kind="ExternalInput").ap()
    out = nc.dram_tensor("out", [128, QB], f32, kind="ExternalOutput").ap()
    with tile.TileContext(nc) as tc:
        _body(tc, out, featT, feat2T, ybc, xb, negt, aexp)
    nc.compile()
    return nc


def host_prep(feat, feat2, temp, labels):
    import ml_dtypes
    feat = np.ascontiguousarray(np.asarray(feat, dtype=np.float32))
    feat2 = np.ascontiguousarray(np.asarray(feat2, dtype=np.float32))
    labels_np = np.asarray(labels).astype(np.int64)
    t = float(np.asarray(temp, dtype=np.float32))

    y_sq = np.einsum("md,md->m", feat2, feat2, dtype=np.float64)
    x_sq = np.einsum("nd,nd->n", feat, feat, dtype=np.float64)
    ybc_full = np.broadcast_to(
        (-0.5 * (y_sq - 128.0)).astype(np.float32)[None, :], (128, M))
    ybc_full = np.ascontiguousarray(ybc_full).astype(ml_dtypes.bfloat16)
    feat2T = np.ascontiguousarray(feat2.T).astype(ml_dtypes.bfloat16)
    negt = np.full((128, 1), -t, dtype=np.float32)
    aexp = np.full((128, 1), -t * A_EXP, dtype=np.float32)

    diff = feat - feat2[labels_np]
    pdist = np.sqrt(np.einsum("nd,nd->n", diff, diff, dtype=np.float64))
    tpd = (t * pdist).astype(np.float64)          # [N], query order

    in_maps = []
    for c in range(N_CORES):
        fs = feat[c * NQ:(c + 1) * NQ]
        xbc = (x_sq[c * NQ:(c + 1) * NQ].reshape(QB, 128).T + 128.0)
        in_maps.append({
            "featT": np.ascontiguousarray(fs.T).astype(ml_dtypes.bfloat16),
            "feat2T": feat2T,
            "ybc": ybc_full,
            "xb": np.ascontiguousarray(xbc).astype(np.float32),
            "negt": negt,
            "aexp": aexp,
        })
    return in_maps, tpd


def finish(per_core_outs, tpd):
    # S[p, b] is sum_j exp(-t*dist) for query q = b*128 + p of that core
    srows = [np.asarray(o, dtype=np.float64).T.reshape(-1)
             for o in per_core_outs]
    S = np.concatenate(srows)                      # [N], query order
    loss = np.log(S) + tpd
    return np.float32(loss.mean())


_PROGRAM = None


def kernel(feat, feat2, temp, labels):
    global _PROGRAM
    if _PROGRAM is None:
        _PROGRAM = build_program()
    in_maps, tpd = host_prep(feat, feat2, temp, labels)
    res = run_bass_kernel_spmd(_PROGRAM, in_maps, core_ids=list(range(N_CORES)))
    return finish([r["out"] for r in res.results], tpd)


# revision 3
# speedup vs baseline: 1.5226x; 1.3257x over previous
"""Trainium2 Bass kernel for nn_CLoss_68521908241007 (retrieval_knn).

Math (per the reference):
  sq_dist[i,j] = ||feat_i||^2 + ||feat2_j||^2 - 2 feat_i . feat2_j
  logits = -temp * sqrt(sq_dist)
  loss = mean_i( logsumexp_j(logits[i,:]) - logits[i, labels_i] )

Sharding: feat rows split across 8 cores (1024 queries each); feat2
replicated.  Each core returns S[p,b] = sum_j exp(-temp*dist) for query
q=b*128+p of its shard; the host finishes with ln(S) + temp*pdist and
the mean.

Per-core pipeline (the whole sq_dist assembles inside the PE):
  - PE fp8 DoubleRow matmul, contraction 256 = two planes:
      plane0: (-2*feat).T fp8e4  x  feat2.T fp8e4   -> -2 x.y
      plane1: ones rows 0..2     x  [yc; ym; yr]    -> +y_sq  (3-row exact
              e4m3 split of y_sq: 16-multiples + 1-multiples + remainder)
    One matmul per (qblock, 512-seg); PSUM accumulates the full sq_dist
    minus x_sq.
  - ACT: dist = Sqrt(psum + x_sq) straight from PSUM (bias = per-partition
    x_sq, so no drain and no separate y_sq add anywhere).
  - exp + row-sum, split to balance ACT vs DVE:
      * A_QBS trailing qblocks: ACT Exp(scale=-temp) with fused accum
      * the rest: DVE Schraudolph exp  w = int32(dist*(-temp*2^23/ln2)+B);
        bits(w) viewed as f32 are exp(-temp*dist); row-sum via
        tensor_reduce on the bitcast view.  (B mean-zero tuned, C=486411.)
  - fp8 dot noise (~+-0.03 on dist) and Schraudolph error (~+-4% per
    element) are pseudo-random across 8192 keys and average out in S;
    measured end-to-end loss error ~5e-4 vs the 2e-2 gate.
"""

import numpy as np
from contextlib import ExitStack

import concourse.bass as bass
import concourse.bacc as bacc
import concourse.mybir as mybir
import concourse.tile as tile
from concourse.bass_utils import run_bass_kernel_spmd

AF = mybir.ActivationFunctionType
ALU = mybir.AluOpType
AX = mybir.AxisListType
f32 = mybir.dt.float32
bf16 = mybir.dt.bfloat16
i32 = mybir.dt.int32
fp8 = mybir.dt.float8e4

N_CORES = 8
N, M, D = 8192, 8192, 128
NQ = N // N_CORES        # queries per core
QB = NQ // 128           # q-blocks per core (8)
KSEG = 512               # keys per matmul
GRP = 4                  # k-segs per psum group (4 banks)
NGRP = (M // KSEG) // GRP  # 4 groups per q-block

A_QBS = 3                # trailing qblocks whose exp+sum runs on ACT
A_EXP = 2.0 ** 23 / np.log(2.0)
B_EXP = float((127 << 23) - 486411)


def _body(tc, out_d, lhs_d, rhs_d, xb_d, negt_d, aexp_d):
    nc = tc.nc
    with ExitStack() as ctx:
        singles = ctx.enter_context(tc.tile_pool(name="singles", bufs=1))
        distp = ctx.enter_context(tc.tile_pool(name="distp", bufs=5))
        psp = ctx.enter_context(tc.tile_pool(name="psp", bufs=2, space="PSUM"))

        # ---- inputs -> SBUF; rhs first, in 8 column chunks so the first
        # matmul can start as soon as chunk 0 lands
        rhs_sb = singles.tile([D, 2, M], fp8)
        NCH = 8
        for c in range(NCH):
            w = M // NCH
            nc.sync.dma_start(out=rhs_sb[:, :, c * w:(c + 1) * w],
                              in_=rhs_d[:, :, c * w:(c + 1) * w])
        lhs_sb = singles.tile([D, 2, NQ], fp8)
        nc.sync.dma_start(out=lhs_sb, in_=lhs_d)
        xb = singles.tile([128, QB], f32)
        nc.sync.dma_start(out=xb, in_=xb_d)
        negt = singles.tile([128, 1], f32)
        nc.sync.dma_start(out=negt, in_=negt_d)
        aexp = singles.tile([128, 1], f32)
        nc.sync.dma_start(out=aexp, in_=aexp_d)

        S = singles.tile([128, QB], f32)
        w_t = singles.tile([128, M], i32)   # Schraudolph scratch (1 buf)
        zrow = singles.tile([128, 1], f32)
        dist_tiles = [None] * QB

        # ---- main stream: per qblock: 16 DoubleRow matmuls + 4 PSUM-read
        # sqrts; Schraudolph qbs' DVE work issues right behind each sqrt.
        for b in range(QB):
            dist_t = distp.tile([128, M], bf16, tag="dist")
            dist_tiles[b] = dist_t
            for g in range(NGRP):
                ps = psp.tile([128, GRP * KSEG], f32, tag="ps")
                for si in range(GRP):
                    s = g * GRP + si
                    nc.tensor.matmul(
                        ps[:, si * KSEG:(si + 1) * KSEG],
                        lhsT=lhs_sb[:, :, b * 128:(b + 1) * 128],
                        rhs=rhs_sb[:, :, s * KSEG:(s + 1) * KSEG],
                        start=True, stop=True,
                        perf_mode=mybir.MatmulPerfMode.DoubleRow)
                nc.scalar.activation(
                    out=dist_t[:, g * GRP * KSEG:(g + 1) * GRP * KSEG],
                    in_=ps, func=AF.Sqrt, bias=xb[:, b:b + 1], scale=1.0)
            if b < QB - A_QBS:
                # Schraudolph exp + reduce on DVE
                nc.vector.tensor_scalar(
                    out=w_t, in0=dist_t, scalar1=aexp[:, 0:1],
                    scalar2=B_EXP, op0=ALU.mult, op1=ALU.add)
                nc.vector.tensor_reduce(
                    out=S[:, b:b + 1], in_=w_t.bitcast(f32),
                    axis=AX.X, op=ALU.add)

        # ---- trailing ACT exps (one table switch, after all sqrts)
        nc.vector.tensor_scalar_mul(zrow, dist_tiles[QB - 1][:, M - 1:M], 0.0)
        for b in range(QB - A_QBS, QB):
            nc.scalar.activation(
                out=dist_tiles[b], in_=dist_tiles[b], func=AF.Exp,
                bias=zrow[:, 0:1], scale=negt[:, 0:1],
                accum_out=S[:, b:b + 1])

        nc.sync.dma_start(out=out_d, in_=S)


def build_program():
    nc = bacc.Bacc("TRN2", target_bir_lowering=False, debug=False,
                   num_devices=N_CORES)
    lhs = nc.dram_tensor("lhs", [D, 2, NQ], fp8, kind="ExternalInput").ap()
    rhs = nc.dram_tensor("rhs", [D, 2, M], fp8, kind="ExternalInput").ap()
    xb = nc.dram_tensor("xb", [128, QB], f32, kind="ExternalInput").ap()
    negt = nc.dram_tensor("negt", [128, 1], f32, kind="ExternalInput").ap()
    aexp = nc.dram_tensor("aexp", [128, 1], f32, kind="ExternalInput").ap()
    out = nc.dram_tensor("out", [128, QB], f32, kind="ExternalOutput").ap()
    with tile.TileContext(nc) as tc:
        _body(tc, out, lhs, rhs, xb, negt, aexp)
    nc.compile()
    return nc


def host_prep(feat, feat2, temp, labels):
    import ml_dtypes
    e4 = ml_dtypes.float8_e4m3
    feat = np.ascontiguousarray(np.asarray(feat, dtype=np.float32))
    feat2 = np.ascontiguousarray(np.asarray(feat2, dtype=np.float32))
    labels_np = np.asarray(labels).astype(np.int64)
    t = float(np.asarray(temp, dtype=np.float32))

    y_sq = np.einsum("md,md->m", feat2, feat2, dtype=np.float64)
    x_sq = np.einsum("nd,nd->n", feat, feat, dtype=np.float64)

    # rhs fp8 [D, 2, M]: plane0 = feat2.T, plane1 rows 0..2 = y_sq split
    rhs = np.zeros((D, 2, M), dtype=e4)
    rhs[:, 0, :] = feat2.T.astype(e4)
    yc = np.floor(y_sq / 16.0) * 16.0
    ym = np.floor(y_sq - yc)
    yr = y_sq - yc - ym
    rhs[0, 1, :] = yc.astype(np.float32).astype(e4)
    rhs[1, 1, :] = ym.astype(np.float32).astype(e4)
    rhs[2, 1, :] = yr.astype(np.float32).astype(e4)

    negt = np.full((128, 1), -t, dtype=np.float32)
    aexp = np.full((128, 1), -t * A_EXP, dtype=np.float32)

    diff = feat - feat2[labels_np]
    pdist = np.sqrt(np.einsum("nd,nd->n", diff, diff, dtype=np.float64))
    tpd = (t * pdist).astype(np.float64)          # [N], query order

    in_maps = []
    for c in range(N_CORES):
        fs = feat[c * NQ:(c + 1) * NQ]
        lhs = np.zeros((D, 2, NQ), dtype=e4)
        lhs[:, 0, :] = (-2.0 * fs.T).astype(e4)
        lhs[0:3, 1, :] = np.ones((3, NQ), dtype=e4)
        xbc = x_sq[c * NQ:(c + 1) * NQ].reshape(QB, 128).T
        in_maps.append({
            "lhs": lhs,
            "rhs": rhs,
            "xb": np.ascontiguousarray(xbc).astype(np.float32),
            "negt": negt,
            "aexp": aexp,
        })
    return in_maps, tpd


def finish(per_core_outs, tpd):
    # S[p, b] is sum_j exp(-t*dist) for query q = b*128 + p of that core
    srows = [np.asarray(o, dtype=np.float64).T.reshape(-1)
             for o in per_core_outs]
    S = np.concatenate(srows)                      # [N], query order
    loss = np.log(S) + tpd
    return np.float32(loss.mean())


_PROGRAM = None


def kernel(feat, feat2, temp, labels):
    global _PROGRAM
    if _PROGRAM is None:
        _PROGRAM = build_program()
    in_maps, tpd = host_prep(feat, feat2, temp, labels)
    res = run_bass_kernel_spmd(_PROGRAM, in_maps, core_ids=list(range(N_CORES)))
    return finish([r["out"] for r in res.results], tpd)


# revision 4
# speedup vs baseline: 1.8148x; 1.1919x over previous
"""Trainium2 Bass kernel for nn_CLoss_68521908241007 (retrieval_knn).

Math (per the reference):
  sq_dist[i,j] = ||feat_i||^2 + ||feat2_j||^2 - 2 feat_i . feat2_j
  logits = -temp * sqrt(sq_dist)
  loss = mean_i( logsumexp_j(logits[i,:]) - logits[i, labels_i] )

Sharding: feat rows split across 8 cores (1024 queries each); feat2
replicated.  Each core returns S[p,b] = sum_j exp(-temp*dist) for query
q=b*128+p of its shard; the host finishes with ln(S) + temp*pdist and
the mean.

Per-core pipeline (PE assembles sq_dist; ACT only sqrts; DVE only exps):
  - PE fp8 DoubleRow matmul, contraction 256 = two planes:
      plane0: (-2*feat).T fp8e4  x  feat2.T fp8e4   -> -2 x.y
      plane1: ones rows 0..2     x  [yc; ym; yr]    -> +y_sq  (3-row exact
              e4m3 split of y_sq)
    One matmul per (qblock, 512-seg); PSUM gets sq_dist minus x_sq.
  - ACT: dist = Sqrt(psum + x_sq) straight from PSUM (bias = per-partition
    x_sq); 32 back-to-back sqrt calls, a single table load, no other ACT
    work -- ACT is the pacing engine at ~1.86us per [128,2048].
  - DVE: 16-bit Schraudolph exp:  w16 = int16(dist*(-temp*2^7/ln2) + B16);
    bits(w16) viewed as bf16 are exp(-temp*dist).  tensor_scalar runs in
    4x mode (16-bit in/out).  Row-sum = 3 bf16 tensor_add folds (2x mode)
    8192->1024 + one small 1x reduce.  ~7.5us/qblock, tracks the sqrt
    stream with no trailing ACT phase.
  - fp8 dot noise (~+-0.03 on dist), Schraudolph error (~+-4%/element),
    and bf16 fold noise are pseudo-random across 8192 keys and average
    out in S; end-to-end loss error ~1e-3 vs the 2e-2 gate (offline
    calibrated, B16 = 127*2^7 - 7.42 mean-zero constant).
"""

import numpy as np
from contextlib import ExitStack

import concourse.bass as bass
import concourse.bacc as bacc
import concourse.mybir as mybir
import concourse.tile as tile
from concourse.bass_utils import run_bass_kernel_spmd

AF = mybir.ActivationFunctionType
ALU = mybir.AluOpType
AX = mybir.AxisListType
f32 = mybir.dt.float32
bf16 = mybir.dt.bfloat16
i16 = mybir.dt.int16
fp8 = mybir.dt.float8e4

N_CORES = 8
N, M, D = 8192, 8192, 128
NQ = N // N_CORES        # queries per core
QB = NQ // 128           # q-blocks per core (8)
KSEG = 512               # keys per matmul
GRP = 4                  # k-segs per psum group (4 banks)
NGRP = (M // KSEG) // GRP  # 4 groups per q-block

A_EXP16 = 2.0 ** 7 / np.log(2.0)
B_EXP16 = float(127 * 2 ** 7) - 7.42


def _body(tc, out_d, lhs_d, rhs_d, xb_d, aexp_d):
    nc = tc.nc
    with ExitStack() as ctx:
        singles = ctx.enter_context(tc.tile_pool(name="singles", bufs=1))
        distp = ctx.enter_context(tc.tile_pool(name="distp", bufs=3))
        foldp = ctx.enter_context(tc.tile_pool(name="foldp", bufs=2))
        psp = ctx.enter_context(tc.tile_pool(name="psp", bufs=2, space="PSUM"))

        # ---- inputs -> SBUF; rhs first, in 8 column chunks so the first
        # matmul can start as soon as chunk 0 lands
        rhs_sb = singles.tile([D, 2, M], fp8)
        NCH = 8
        for c in range(NCH):
            w = M // NCH
            nc.sync.dma_start(out=rhs_sb[:, :, c * w:(c + 1) * w],
                              in_=rhs_d[:, :, c * w:(c + 1) * w])
        lhs_sb = singles.tile([D, 2, NQ], fp8)
        nc.sync.dma_start(out=lhs_sb, in_=lhs_d)
        xb = singles.tile([128, QB], f32)
        nc.sync.dma_start(out=xb, in_=xb_d)
        aexp = singles.tile([128, 1], f32)
        nc.sync.dma_start(out=aexp, in_=aexp_d)

        S = singles.tile([128, QB], f32)
        w_t = singles.tile([128, M], i16)   # Schraudolph bits (1 buf)

        # ---- main stream: per qblock: 16 DoubleRow matmuls + 4 PSUM-read
        # sqrts; DVE exp+fold chain issues right behind each qblock.
        for b in range(QB):
            dist_t = distp.tile([128, M], bf16, tag="dist")
            for g in range(NGRP):
                ps = psp.tile([128, GRP * KSEG], f32, tag="ps")
                for si in range(GRP):
                    s = g * GRP + si
                    nc.tensor.matmul(
                        ps[:, si * KSEG:(si + 1) * KSEG],
                        lhsT=lhs_sb[:, :, b * 128:(b + 1) * 128],
                        rhs=rhs_sb[:, :, s * KSEG:(s + 1) * KSEG],
                        start=True, stop=True,
                        perf_mode=mybir.MatmulPerfMode.DoubleRow)
                nc.scalar.activation(
                    out=dist_t[:, g * GRP * KSEG:(g + 1) * GRP * KSEG],
                    in_=ps, func=AF.Sqrt, bias=xb[:, b:b + 1], scale=1.0)
            # Schraudolph exp (4x) + bf16 fold chain (2x) + small reduce
            nc.vector.tensor_scalar(
                out=w_t, in0=dist_t, scalar1=aexp[:, 0:1],
                scalar2=B_EXP16, op0=ALU.mult, op1=ALU.add)
            e_v = w_t.bitcast(bf16)
            v4 = foldp.tile([128, M // 2], bf16, tag="v4")
            nc.vector.tensor_add(v4, e_v[:, :M // 2], e_v[:, M // 2:])
            v2 = foldp.tile([128, M // 4], bf16, tag="v2")
            nc.vector.tensor_add(v2, v4[:, :M // 4], v4[:, M // 4:])
            v1 = foldp.tile([128, M // 8], bf16, tag="v1")
            nc.vector.tensor_add(v1, v2[:, :M // 8], v2[:, M // 8:])
            nc.vector.tensor_reduce(
                out=S[:, b:b + 1], in_=v1, axis=AX.X, op=ALU.add)

        nc.sync.dma_start(out=out_d, in_=S)


def build_program():
    nc = bacc.Bacc("TRN2", target_bir_lowering=False, debug=False,
                   num_devices=N_CORES)
    lhs = nc.dram_tensor("lhs", [D, 2, NQ], fp8, kind="ExternalInput").ap()
    rhs = nc.dram_tensor("rhs", [D, 2, M], fp8, kind="ExternalInput").ap()
    xb = nc.dram_tensor("xb", [128, QB], f32, kind="ExternalInput").ap()
    aexp = nc.dram_tensor("aexp", [128, 1], f32, kind="ExternalInput").ap()
    out = nc.dram_tensor("out", [128, QB], f32, kind="ExternalOutput").ap()
    with tile.TileContext(nc) as tc:
        _body(tc, out, lhs, rhs, xb, aexp)
    nc.compile()
    return nc


def host_prep(feat, feat2, temp, labels):
    import ml_dtypes
    e4 = ml_dtypes.float8_e4m3
    feat = np.ascontiguousarray(np.asarray(feat, dtype=np.float32))
    feat2 = np.ascontiguousarray(np.asarray(feat2, dtype=np.float32))
    labels_np = np.asarray(labels).astype(np.int64)
    t = float(np.asarray(temp, dtype=np.float32))

    y_sq = np.einsum("md,md->m", feat2, feat2, dtype=np.float64)
    x_sq = np.einsum("nd,nd->n", feat, feat, dtype=np.float64)

    # rhs fp8 [D, 2, M]: plane0 = feat2.T, plane1 rows 0..2 = y_sq split
    rhs = np.zeros((D, 2, M), dtype=e4)
    rhs[:, 0, :] = feat2.T.astype(e4)
    yc = np.floor(y_sq / 16.0) * 16.0
    ym = np.floor(y_sq - yc)
    yr = y_sq - yc - ym
    rhs[0, 1, :] = yc.astype(np.float32).astype(e4)
    rhs[1, 1, :] = ym.astype(np.float32).astype(e4)
    rhs[2, 1, :] = yr.astype(np.float32).astype(e4)

    aexp = np.full((128, 1), -t * A_EXP16, dtype=np.float32)

    diff = feat - feat2[labels_np]
    pdist = np.sqrt(np.einsum("nd,nd->n", diff, diff, dtype=np.float64))
    tpd = (t * pdist).astype(np.float64)          # [N], query order

    in_maps = []
    for c in range(N_CORES):
        fs = feat[c * NQ:(c + 1) * NQ]
        lhs = np.zeros((D, 2, NQ), dtype=e4)
        lhs[:, 0, :] = (-2.0 * fs.T).astype(e4)
        lhs[0:3, 1, :] = np.ones((3, NQ), dtype=e4)
        xbc = x_sq[c * NQ:(c + 1) * NQ].reshape(QB, 128).T
        in_maps.append({
            "lhs": lhs,
            "rhs": rhs,
            "xb": np.ascontiguousarray(xbc).astype(np.float32),
            "aexp": aexp,
        })
    return in_maps, tpd


def finish(per_core_outs, tpd):
    # S[p, b] is sum_j exp(-t*dist) for query q = b*128 + p of that core
    srows = [np.asarray(o, dtype=np.float64).T.reshape(-1)
             for o in per_core_outs]
    S = np.concatenate(srows)                      # [N], query order
    loss = np.log(S) + tpd
    return np.float32(loss.mean())


_PROGRAM = None


def kernel(feat, feat2, temp, labels):
    global _PROGRAM
    if _PROGRAM is None:
        _PROGRAM = build_program()
    in_maps, tpd = host_prep(feat, feat2, temp, labels)
    res = run_bass_kernel_spmd(_PROGRAM, in_maps, core_ids=list(range(N_CORES)))
    return finish([r["out"] for r in res.results], tpd)


# revision 10
# speedup vs baseline: 1.8772x; 1.0344x over previous
"""Trainium2 Bass kernel for nn_CLoss_68521908241007 (retrieval_knn).

Math (per the reference):
  sq_dist[i,j] = ||feat_i||^2 + ||feat2_j||^2 - 2 feat_i . feat2_j
  logits = -temp * sqrt(sq_dist)
  loss = mean_i( logsumexp_j(logits[i,:]) - logits[i, labels_i] )

Sharding: feat rows split across 8 cores (1024 queries each); feat2
replicated.  Each core returns S[p,b] = sum_j exp(-temp*dist) for query
q=b*128+p of its shard; the host finishes with ln(S) + temp*pdist and
the mean.

Per-core pipeline (PE assembles sq_dist; ACT only sqrts; DVE only exps):
  - PE fp8 DoubleRow matmul, contraction 256 = two planes:
      plane0: (-2*feat).T fp8e4  x  feat2.T fp8e4   -> -2 x.y
      plane1: ones rows 0..2     x  [yc; ym; yr]    -> +y_sq  (3-row exact
              e4m3 split of y_sq)
    One matmul per (qblock, 512-seg); PSUM gets sq_dist minus x_sq.
  - ACT: dist = Sqrt(psum + x_sq) straight from PSUM (bias = per-partition
    x_sq); 32 back-to-back sqrt calls, a single table load, no other ACT
    work -- ACT is the pacing engine at ~1.86us per [128,2048].
  - DVE: 16-bit Schraudolph exp:  w16 = int16(dist*(-temp*2^7/ln2) + B16);
    bits(w16) viewed as bf16 are exp(-temp*dist).  tensor_scalar runs in
    4x mode (16-bit in/out).  Row-sum = 3 bf16 tensor_add folds (2x mode)
    8192->1024 + one small 1x reduce.  ~7.5us/qblock, tracks the sqrt
    stream with no trailing ACT phase.
  - fp8 dot noise (~+-0.03 on dist), Schraudolph error (~+-4%/element),
    and bf16 fold noise are pseudo-random across 8192 keys and average
    out in S; end-to-end loss error ~1e-3 vs the 2e-2 gate (offline
    calibrated, B16 = 127*2^7 - 7.42 mean-zero constant).
"""

import numpy as np
from contextlib import ExitStack

import concourse.bass as bass
import concourse.bacc as bacc
import concourse.mybir as mybir
import concourse.tile as tile
from concourse.bass_utils import run_bass_kernel_spmd

AF = mybir.ActivationFunctionType
ALU = mybir.AluOpType
AX = mybir.AxisListType
f32 = mybir.dt.float32
bf16 = mybir.dt.bfloat16
i16 = mybir.dt.int16
fp8 = mybir.dt.float8e4

N_CORES = 8
N, M, D = 8192, 8192, 128
NQ = N // N_CORES        # queries per core
QB = NQ // 128           # q-blocks per core (8)
KSEG = 512               # keys per matmul
GRP = 4                  # k-segs per psum group (4 banks)
NGRP = (M // KSEG) // GRP  # 4 groups per q-block

A_EXP16 = 2.0 ** 7 / np.log(2.0)
B_EXP16 = float(127 * 2 ** 7) - 7.42
NCH = 8                  # rhs dma chunks (separate contiguous dram tensors)


def _body(tc, out_d, lhs_d, rhs_d, xb_d, aexp_d):
    nc = tc.nc
    with ExitStack() as ctx:
        singles = ctx.enter_context(tc.tile_pool(name="singles", bufs=1))
        distp = ctx.enter_context(tc.tile_pool(name="distp", bufs=3))
        foldp = ctx.enter_context(tc.tile_pool(name="foldp", bufs=2))
        psp = ctx.enter_context(tc.tile_pool(name="psp", bufs=2, space="PSUM"))

        # ---- inputs -> SBUF; rhs chunk 0 + lhs first so the first matmul
        # starts early (each chunk is its own contiguous dram tensor)
        rhs_sb = singles.tile([D, 2, M], fp8)
        w = M // NCH
        nc.sync.dma_start(out=rhs_sb[:, :, 0:w], in_=rhs_d[0])
        lhs_sb = singles.tile([D, 2, NQ], fp8)
        nc.sync.dma_start(out=lhs_sb, in_=lhs_d)
        for c in range(1, NCH):
            nc.sync.dma_start(out=rhs_sb[:, :, c * w:(c + 1) * w],
                              in_=rhs_d[c])
        xb = singles.tile([128, QB], f32)
        nc.sync.dma_start(out=xb, in_=xb_d)
        aexp = singles.tile([128, 1], f32)
        nc.sync.dma_start(out=aexp, in_=aexp_d)

        S = singles.tile([128, QB], f32)
        w_t = singles.tile([128, M], i16)   # Schraudolph bits (1 buf)

        # ---- main stream: per qblock: 16 DoubleRow matmuls + 4 PSUM-read
        # sqrts; DVE exp+fold chain issues right behind each qblock.
        GW = GRP * KSEG  # 2048, one psum group's width
        for b in range(QB):
            dist_t = distp.tile([128, M], bf16, tag="dist")
            e_v = w_t.bitcast(bf16)
            for g in range(NGRP):
                ps = psp.tile([128, GW], f32, tag="ps")
                for si in range(GRP):
                    s = g * GRP + si
                    nc.tensor.matmul(
                        ps[:, si * KSEG:(si + 1) * KSEG],
                        lhsT=lhs_sb[:, :, b * 128:(b + 1) * 128],
                        rhs=rhs_sb[:, :, s * KSEG:(s + 1) * KSEG],
                        start=True, stop=True,
                        perf_mode=mybir.MatmulPerfMode.DoubleRow)
                nc.scalar.activation(
                    out=dist_t[:, g * GW:(g + 1) * GW],
                    in_=ps, func=AF.Sqrt, bias=xb[:, b:b + 1], scale=1.0)
                # Schraudolph exp (4x) per chunk, right behind the sqrt
                nc.vector.tensor_scalar(
                    out=w_t[:, g * GW:(g + 1) * GW],
                    in0=dist_t[:, g * GW:(g + 1) * GW],
                    scalar1=aexp[:, 0:1], scalar2=B_EXP16,
                    op0=ALU.mult, op1=ALU.add)
            # pairwise bf16 fold tree (2x) + small reduce; only ~3us of
            # this trails the final sqrt of the qblock
            va = foldp.tile([128, GW], bf16, tag="va")
            nc.vector.tensor_add(va, e_v[:, 0:GW], e_v[:, GW:2 * GW])
            vb = foldp.tile([128, GW], bf16, tag="vb")
            nc.vector.tensor_add(vb, e_v[:, 2 * GW:3 * GW], e_v[:, 3 * GW:])
            vc = foldp.tile([128, GW], bf16, tag="vc")
            nc.vector.tensor_add(vc, va, vb)
            v1 = foldp.tile([128, GW // 2], bf16, tag="v1")
            nc.vector.tensor_add(v1, vc[:, :GW // 2], vc[:, GW // 2:])
            nc.vector.tensor_reduce(
                out=S[:, b:b + 1], in_=v1, axis=AX.X, op=ALU.add)

        nc.sync.dma_start(out=out_d, in_=S)


def build_program():
    nc = bacc.Bacc("TRN2", target_bir_lowering=False, debug=False,
                   num_devices=N_CORES)
    lhs = nc.dram_tensor("lhs", [D, 2, NQ], fp8, kind="ExternalInput").ap()
    rhs = [nc.dram_tensor(f"rhs{c}", [D, 2, M // NCH], fp8,
                          kind="ExternalInput").ap() for c in range(NCH)]
    xb = nc.dram_tensor("xb", [128, QB], f32, kind="ExternalInput").ap()
    aexp = nc.dram_tensor("aexp", [128, 1], f32, kind="ExternalInput").ap()
    out = nc.dram_tensor("out", [128, QB], f32, kind="ExternalOutput").ap()
    with tile.TileContext(nc) as tc:
        _body(tc, out, lhs, rhs, xb, aexp)
    nc.compile()
    return nc


def host_prep(feat, feat2, temp, labels):
    import ml_dtypes
    e4 = ml_dtypes.float8_e4m3
    feat = np.ascontiguousarray(np.asarray(feat, dtype=np.float32))
    feat2 = np.ascontiguousarray(np.asarray(feat2, dtype=np.float32))
    labels_np = np.asarray(labels).astype(np.int64)
    t = float(np.asarray(temp, dtype=np.float32))

    y_sq = np.einsum("md,md->m", feat2, feat2, dtype=np.float64)
    x_sq = np.einsum("nd,nd->n", feat, feat, dtype=np.float64)

    # rhs fp8 [D, 2, M]: plane0 = feat2.T, plane1 rows 0..2 = y_sq split;
    # shipped as NCH contiguous column-chunk tensors
    rhs = np.zeros((D, 2, M), dtype=e4)
    rhs[:, 0, :] = feat2.T.astype(e4)
    yc = np.floor(y_sq / 16.0) * 16.0
    ym = np.floor(y_sq - yc)
    yr = y_sq - yc - ym
    rhs[0, 1, :] = yc.astype(np.float32).astype(e4)
    rhs[1, 1, :] = ym.astype(np.float32).astype(e4)
    rhs[2, 1, :] = yr.astype(np.float32).astype(e4)
    w = M // NCH
    rhs_chunks = {f"rhs{c}": np.ascontiguousarray(rhs[:, :, c * w:(c + 1) * w])
                  for c in range(NCH)}

    aexp = np.full((128, 1), -t * A_EXP16, dtype=np.float32)

    diff = feat - feat2[labels_np]
    pdist = np.sqrt(np.einsum("nd,nd->n", diff, diff, dtype=np.float64))
    tpd = (t * pdist).astype(np.float64)          # [N], query order

    in_maps = []
    for c in range(N_CORES):
        fs = feat[c * NQ:(c + 1) * NQ]
        lhs = np.zeros((D, 2, NQ), dtype=e4)
        lhs[:, 0, :] = (-2.0 * fs.T).astype(e4)
        lhs[0:3, 1, :] = np.ones((3, NQ), dtype=e4)
        xbc = x_sq[c * NQ:(c + 1) * NQ].reshape(QB, 128).T
        in_maps.append({
            "lhs": lhs,
            **rhs_chunks,
            "xb": np.ascontiguousarray(xbc).astype(np.float32),
            "aexp": aexp,
        })
    return in_maps, tpd


def finish(per_core_outs, tpd):
    # S[p, b] is sum_j exp(-t*dist) for query q = b*128 + p of that core
    srows = [np.asarray(o, dtype=np.float64).T.reshape(-1)
             for o in per_core_outs]
    S = np.concatenate(srows)                      # [N], query order
    loss = np.log(S) + tpd
    return np.float32(loss.mean())


_PROGRAM = None


def kernel(feat, feat2, temp, labels):
    global _PROGRAM
    if _PROGRAM is None:
        _PROGRAM = build_program()
    in_maps, tpd = host_prep(feat, feat2, temp, labels)
    res = run_bass_kernel_spmd(_PROGRAM, in_maps, core_ids=list(range(N_CORES)))
    return finish([r["out"] for r in res.results], tpd)
